# revision 25
# baseline (speedup 1.0000x reference)
"""MoE top-1 routing kernel for Trainium2 (8 NeuronCores, expert-parallel).

Strategy:
  - Host: gate (tiny [T,1024]@[1024,8] matmul), softmax, top-1 routing,
    group tokens by expert (one expert per core), pad to capacity C.
  - Device (per core e): yT = w2[e].T @ gelu(w1[e].T @ xT_e + b1[e])
    with fp32 storage and float32r matmuls (full PE rate).
  - Host: scatter outputs back to token order, scale by gate weight,
    add b2 (zeros in practice, handled exactly anyway).

Shapes (hardcoded): x [4,2048,1024], 8 experts, top-1, d=1024, h=4096.
"""

import sys

for _p in ("/opt/trn_rl_repo",):
    if _p not in sys.path:
        sys.path.append(_p)

import numpy as np

D = 1024
H = 4096
E = 8
NP = 128  # partitions

KD = D // NP   # 8 contraction chunks for mm1
NJB = 8        # j blocks
JW = H // NJB  # 512 j columns per block
NJJ = JW // NP  # 4 sub-tiles of 128 j per block
NDC = D // NP  # 8 output row blocks

_cache = {}


def _ctile_split(C):
    """Split C into pieces in [256, 512], all multiples of 4 (fp32r ISA
    restriction on the moving operand width; full rate needs >=256)."""
    assert C % 4 == 0
    out = []
    r = C
    while r > 0:
        if r <= 512:
            out.append(r)
            break
        if r <= 768:
            a = -(-r // 8) * 4
            out.append(a)
            out.append(r - a)
            break
        out.append(512)
        r -= 512
    assert all(256 <= p <= 512 and p % 4 == 0 for p in out) and sum(out) == C, (C, out)
    return out


def _build(C, act=None):
    """Build + compile the per-core Bass kernel for capacity C tokens."""
    from contextlib import ExitStack

    import concourse.bass as bass  # noqa: F401
    import concourse.tile as tile
    from concourse import bacc, mybir

    f32 = mybir.dt.float32
    f32r = mybir.dt.float32r
    GELU = (
        mybir.ActivationFunctionType.Gelu
        if act is None
        else getattr(mybir.ActivationFunctionType, act)
    )

    ctiles = []
    c0 = 0
    for cw in _ctile_split(C):
        ctiles.append((c0, cw))
        c0 += cw

    nc = bacc.Bacc("TRN2", target_bir_lowering=False, debug=False, num_devices=E)
    xT_d = nc.dram_tensor("xT", [D, C], f32r, kind="ExternalInput").ap()
    w1_d = nc.dram_tensor("w1", [D, H], f32r, kind="ExternalInput").ap()
    b1_d = nc.dram_tensor("b1t", [NP, H // NP], f32, kind="ExternalInput").ap()
    w2_d = nc.dram_tensor("w2", [H, D], f32r, kind="ExternalInput").ap()
    yT_d = nc.dram_tensor("yT", [D, C], f32, kind="ExternalOutput").ap()

    with tile.TileContext(nc) as tc, ExitStack() as ctx:
        xp = ctx.enter_context(tc.tile_pool(name="x", bufs=1))
        w1p = ctx.enter_context(tc.tile_pool(name="w1", bufs=2))
        w2p = ctx.enter_context(tc.tile_pool(name="w2", bufs=2))
        hp = ctx.enter_context(tc.tile_pool(name="h", bufs=2))
        yp = ctx.enter_context(tc.tile_pool(name="y", bufs=1))
        bp = ctx.enter_context(tc.tile_pool(name="b", bufs=1))
        ps1 = ctx.enter_context(tc.tile_pool(name="ps1", bufs=4, space="PSUM"))
        ps2 = ctx.enter_context(tc.tile_pool(name="ps2", bufs=4, space="PSUM"))

        # Resident tensors: xT (all of it), b1 tiles, yT accumulator.
        b1t = bp.tile([NP, H // NP], f32)
        xt = xp.tile([NP, KD * C], f32r)

        # PE warmup: chew zero matmuls while the first input DMAs land, so
        # the tensor engine is at full clock when real work starts (HAM).
        import os as _os
        n_warm = int(_os.environ.get("KWARM", "32"))
        if n_warm:
            warm = bp.tile([NP, 256], f32r, tag="warm")
            nc.vector.memzero(warm[:])
            for _ in range(n_warm):
                wps = ps2.tile([NP, 256], f32, tag="ps2")
                nc.tensor.matmul(
                    wps[:], warm[:, :NP], warm[:], start=True, stop=True
                )

        def dma_xt(c0, cw):
            nc.sync.dma_start(
                xt[:].rearrange("p (k c) -> p k c", k=KD)[:, :, c0 : c0 + cw],
                xT_d.rearrange("(k p) c -> p k c", p=NP)[:, :, c0 : c0 + cw],
            )

        # first ctile piece of xT first: the opening matmul group needs only
        # this (2.1 MB) plus one w1 jj-quarter (0.5 MB)
        dma_xt(*ctiles[0])
        nc.sync.dma_start(b1t[:], b1_d[:])
        yt = yp.tile([NP, NDC * C], f32)

        for jb in range(NJB):
            # Stream this j-block's weights: w1[:, jb*512:(jb+1)*512] and
            # w2[jb*512:(jb+1)*512, :]. First block's w1 is split per jj so
            # the opening matmul group only waits on ~2.6 MB of DMA.
            w1t = w1p.tile([NP, KD * JW], f32r)
            if jb == 0:
                for jj in range(NJJ):
                    nc.sync.dma_start(
                        w1t[:].rearrange("p (k j) -> p k j", k=KD)[
                            :, :, jj * NP : (jj + 1) * NP
                        ],
                        w1_d[:, jb * JW + jj * NP : jb * JW + (jj + 1) * NP].rearrange(
                            "(k p) j -> p k j", p=NP
                        ),
                    )
                for (c0, cw) in ctiles[1:]:
                    dma_xt(c0, cw)
            else:
                nc.sync.dma_start(
                    w1t[:].rearrange("p (k j) -> p k j", k=KD),
                    w1_d[:, jb * JW : (jb + 1) * JW].rearrange(
                        "(k p) j -> p k j", p=NP
                    ),
                )
            w2t = w2p.tile([NP, NJJ * D], f32r)
            nc.sync.dma_start(
                w2t[:].rearrange("p (j d) -> p j d", j=NJJ),
                w2_d[jb * JW : (jb + 1) * JW, :].rearrange("(j p) d -> p j d", p=NP),
            )

            # mm1: h[j, c] = gelu(sum_d w1[d, j] * xT[d, c] + b1[j])
            ht = hp.tile([NP, NJJ * C], f32r)
            for (jj, (c0, cw)) in [
                (jj, ct) for jj in range(NJJ) for ct in ctiles
            ]:
                ps = ps1.tile([NP, cw], f32, tag="ps1")
                for kd in range(KD):
                    nc.tensor.matmul(
                        ps[:],
                        w1t[:, kd * JW + jj * NP : kd * JW + (jj + 1) * NP],
                        xt[:, kd * C + c0 : kd * C + c0 + cw],
                        start=(kd == 0),
                        stop=(kd == KD - 1),
                    )
                nc.scalar.activation(
                    ht[:, jj * C + c0 : jj * C + c0 + cw],
                    ps[:],
                    GELU,
                    bias=b1t[:, jb * NJJ + jj : jb * NJJ + jj + 1],
                )

            # mm2: yT[dc, c] += sum_{j in jb} w2[j, dc] * h[j, c]
            for dc in range(NDC):
                for (c0, cw) in ctiles:
                    ps = ps2.tile([NP, cw], f32, tag="ps2")
                    for jj in range(NJJ):
                        nc.tensor.matmul(
                            ps[:],
                            w2t[:, jj * D + dc * NP : jj * D + (dc + 1) * NP],
                            ht[:, jj * C + c0 : jj * C + c0 + cw],
                            start=(jj == 0),
                            stop=(jj == NJJ - 1),
                        )
                    dst = yt[:, dc * C + c0 : dc * C + c0 + cw]
                    if jb == 0:
                        nc.vector.tensor_copy(dst, ps[:])
                    else:
                        nc.vector.tensor_add(dst, dst, ps[:])
                # stream each output row-block out as soon as it's final
                if jb == NJB - 1:
                    nc.sync.dma_start(
                        yT_d[dc * NP : (dc + 1) * NP, :],
                        yt[:, dc * C : (dc + 1) * C],
                    )

    nc.compile()
    return nc


def _get_nc(C, act=None):
    key = (C, act)
    if key not in _cache:
        _cache[key] = _build(C, act)
    return _cache[key]


def _route(xf, gate_w, gate_b):
    """Host-side top-1 gate: returns (expert idx [T], gate weight [T])."""
    logits = xf @ gate_w + gate_b
    m = logits.max(-1, keepdims=True)
    ex = np.exp(logits - m)
    pb = ex / ex.sum(-1, keepdims=True)
    idx = logits.argmax(-1)
    wgt = pb[np.arange(pb.shape[0]), idx]
    return idx, wgt


_jit_cache = {}


def _run(nc, in_maps):
    """Execute nc on the 8 cores via PJRT, caching the jitted executable
    across calls (run_bass_via_pjrt re-traces jax on every invocation)."""
    import jax
    from jax.sharding import Mesh, PartitionSpec
    from jax.experimental.shard_map import shard_map
    from concourse import bass2jax, mybir

    key = id(nc)
    if key not in _jit_cache:
        bass2jax.install_neuronx_cc_hook()
        pid_name = nc.partition_id_tensor.name if nc.partition_id_tensor else None
        in_names, out_names, out_avals = [], [], []
        for alloc in nc.m.functions[0].allocations:
            if not isinstance(alloc, mybir.MemoryLocationSet):
                continue
            name = alloc.memorylocations[0].name
            if alloc.kind == "ExternalInput":
                if name != pid_name:
                    in_names.append(name)
            elif alloc.kind == "ExternalOutput":
                out_names.append(name)
                out_avals.append(
                    jax.core.ShapedArray(
                        tuple(alloc.tensor_shape), mybir.dt.np(alloc.dtype)
                    )
                )
        n_params = len(in_names)
        all_names = in_names + out_names
        if pid_name is not None:
            all_names = all_names + [pid_name]

        def _body(*args):
            operands = list(args)
            if pid_name is not None:
                operands.append(bass2jax.partition_id_tensor())
            return tuple(
                bass2jax._bass_exec_p.bind(
                    *operands,
                    out_avals=tuple(out_avals),
                    in_names=tuple(all_names),
                    out_names=tuple(out_names),
                    lowering_input_output_aliases=(),
                    sim_require_finite=True,
                    sim_require_nnan=True,
                    nc=nc,
                )
            )

        mesh = Mesh(np.asarray(jax.devices()[:E]), ("core",))
        nio = n_params + len(out_names)
        sharded = jax.jit(
            shard_map(
                _body,
                mesh=mesh,
                in_specs=(PartitionSpec("core"),) * nio,
                out_specs=(PartitionSpec("core"),) * len(out_names),
                check_rep=False,
            ),
            donate_argnums=tuple(range(n_params, nio)),
            keep_unused=True,
        )
        _jit_cache[key] = (sharded, in_names, out_names, out_avals)

    sharded, in_names, out_names, out_avals = _jit_cache[key]
    concat_in = [
        np.concatenate([np.asarray(m[name]) for m in in_maps], axis=0)
        for name in in_names
    ]
    concat_zeros = [
        np.zeros((E * av.shape[0], *av.shape[1:]), av.dtype) for av in out_avals
    ]
    outs = sharded(*concat_in, *concat_zeros)
    return [
        {
            name: np.asarray(outs[i]).reshape(E, *out_avals[i].shape)[c]
            for i, name in enumerate(out_names)
        }
        for c in range(E)
    ]


def kernel(x, gate_w, gate_b, w1, b1, w2, b2):

    x = np.asarray(x, np.float32)
    gate_w = np.asarray(gate_w, np.float32)
    gate_b = np.asarray(gate_b, np.float32)
    w1 = np.asarray(w1, np.float32)
    b1 = np.asarray(b1, np.float32)
    w2 = np.asarray(w2, np.float32)
    b2 = np.asarray(b2, np.float32)

    b, s, d = x.shape
    T = b * s
    xf = x.reshape(T, d)

    idx, wgt = _route(xf, gate_w, gate_b)

    tids_all = [np.nonzero(idx == e)[0] for e in range(E)]
    maxc = max(len(t) for t in tids_all)
    # SBUF budget caps per-invocation capacity; chunk tokens if the routing
    # is pathologically imbalanced (never happens for the reference inputs)
    CMAX = 1344
    n_chunks = max(1, -(-maxc // CMAX))

    out = np.empty((T, D), np.float32)
    for ci in range(n_chunks):
        tids = [t[ci * CMAX : (ci + 1) * CMAX] for t in tids_all]
        mc = max(len(t) for t in tids)
        # capacity: multiple of 4 (fp32r ISA), >= 512 so ctile pieces land
        # in [256, 512]
        C = max(512, -(-mc // 4) * 4)

        nc = _get_nc(C)

        in_maps = []
        for e in range(E):
            xT = np.zeros((D, C), np.float32)
            n = len(tids[e])
            xT[:, :n] = xf[tids[e]].T
            in_maps.append(
                {
                    "xT": xT,
                    "w1": np.ascontiguousarray(w1[e]),
                    "b1t": np.ascontiguousarray(b1[e].reshape(H // NP, NP).T),
                    "w2": np.ascontiguousarray(w2[e]),
                }
            )

        res = _run(nc, in_maps)

        for e in range(E):
            n = len(tids[e])
            if n:
                y = res[e]["yT"][:, :n].T  # [n, D]
                out[tids[e]] = wgt[tids[e], None] * (y + b2[e])
    return out.reshape(b, s, d)
